# revision 38
# baseline (speedup 1.0000x reference)
"""Multi-head attention (B=4, S=2048, D=1024, H=16) on 8 trn2 NeuronCores.

Sharding: B x H grid -- core c owns batch c//2 and head-group c%2 (8 heads
= 512 model dims).  Each core computes q/k/v projections for its 512 dims
over its batch's 2048 tokens, local attention for its 8 heads, and a
partial out-projection against its 512 rows of Wo.  Host sums the two
partial outputs per batch (the tensor-parallel all-reduce, done at gather
time) and stacks batches.  vs head-only sharding this cuts per-core DMA
from ~69MB to ~16MB; on-chip FLOPs are identical.

Per-core kernel (bf16 operands, fp32 PSUM):
  x_sb   [128, 8, 2048]  x^T (contraction dim on partitions), bf16 from host.
  q/k    [128, 4, 2048]  dim-major (head h = dim-tile h//2, rows (h%2)*64+).
  v_sb   [128, 16, 8, 65] token-major (128 tokens per k-tile on partitions)
         computed directly by x-stationary projection matmuls -- no PE
         transposes -- with a constant ones column per head so attn@v gives
         numerator + softmax denominator in one accumulation group.
  scores [128 k, 512 q] one matmul per (head, ktile, qchunk); exp on ACT
         (scale=1/8 fused) straight to bf16 SBUF tiles.
  division: DVE reciprocal of the denominator row, broadcast across 64
         partitions via a 1-row f32r matmul, DVE multiply into outT.
  oproj  outT [128, 4, 2048] @ wo_sb -> [128 tok, 1024] psum, DVE evict,
         DMA out fp32.

Scheduling: ACT exp (~280us) and PE streaming (~330us) are the co-rooflines;
everything is emitted as self-contained PE work units (projection groups,
attn@v accumulation groups, out-proj groups) hand-paced between the score
matmuls so the PE never idles (p-state stays 2.4GHz) while ACT stays fed
through a 4-deep PSUM score window.  PSUM budget: 4 score banks + 2 attn@v
accumulators + 2 rotating gemm banks = 8.
"""
import os
import sys

sys.path.insert(0, "/opt/trn_rl_repo")

import numpy as np

import concourse.mybir as mybir
import concourse.tile as tile
from concourse import bacc
from concourse._compat import with_exitstack
from concourse.bass_utils import run_bass_kernel_spmd
from contextlib import ExitStack

B, S, D, H = 4, 2048, 1024, 16
HD = D // H              # 64
P = 128
NCORES = 8
NG = 2                   # head-groups (tensor-parallel degree)
HG = H // NG             # 8 heads per core
GD = HG * HD             # 512 group dims
NDT = GD // P            # 4 dim-tiles of q/k/v per core
CDT = D // P             # 8 contraction tiles
KT = S // P              # 16 k-tiles
QC = 512                 # q-chunk / matmul free dim (one psum bank)
NQC = S // QC            # 4
TC = 512                 # projection token chunk
NTC = S // TC            # 4
VE = HD + 1              # v + ones column
EXP_SCALE = float(1.0 / np.sqrt(HD))

f32 = mybir.dt.float32
f32r = mybir.dt.float32r
bf16 = mybir.dt.bfloat16

LAST_EXEC_TIME_NS = None
_CACHED_NC = None
CURRENT_LABEL = ["init"]


def _lbl(s):
    CURRENT_LABEL[0] = s


@with_exitstack
def _mha_kernel(ctx: ExitStack, tc_: tile.TileContext, ins, outs):
    nc = tc_.nc
    xT_d, wqT_d, wkT_d, wvT_d, woT_d, ones16_d = ins
    out_d = outs[0]

    const = ctx.enter_context(tc_.tile_pool(name="const", bufs=1))
    wpool = ctx.enter_context(tc_.tile_pool(name="wpool", bufs=1))
    xpool = ctx.enter_context(tc_.tile_pool(name="xpool", bufs=1))
    qkpool = ctx.enter_context(tc_.tile_pool(name="qkpool", bufs=1))
    vpool = ctx.enter_context(tc_.tile_pool(name="vpool", bufs=1))
    epool = ctx.enter_context(tc_.tile_pool(name="epool", bufs=20))
    rpool = ctx.enter_context(tc_.tile_pool(name="rpool", bufs=2))
    ospool = ctx.enter_context(tc_.tile_pool(name="ospool", bufs=2))

    # PSUM: 2x2 score banks (kt-pair fused exp) + 4 quarter-bank attn@v
    # accumulators (2 banks) + 2 gemm banks = 8
    spool = ctx.enter_context(tc_.tile_pool(name="spool", bufs=2, space="PSUM"))
    apool = ctx.enter_context(tc_.tile_pool(name="apool", bufs=2, space="PSUM"))
    gpool = ctx.enter_context(tc_.tile_pool(name="gpool", bufs=2, space="PSUM"))

    ones_sb = const.tile([P, P], bf16, tag="ones_sb")

    x_sb = xpool.tile([P, CDT * S], bf16, tag="x")
    wq = wpool.tile([P, CDT * GD], bf16, tag="wq")
    wk = wpool.tile([P, CDT * GD], bf16, tag="wk")
    wv = wpool.tile([P, CDT * GD], bf16, tag="wv")
    wo = wpool.tile([P, NDT * D], bf16, tag="wo")
    qT = qkpool.tile([P, NDT * S], bf16, tag="qT")
    kT = qkpool.tile([P, NDT * S], bf16, tag="kT")
    outT = qkpool.tile([P, NDT * S], bf16, tag="outT")
    v_sb = vpool.tile([P, KT * HG * VE], bf16, tag="v")
    v_r = v_sb[:].rearrange("p (k h e) -> p k h e", k=KT, h=HG)

    # All DRAM tensors are host-prearranged SBUF partition images: every
    # DMA is 128 contiguous descriptors.  x chunks are split into
    # contraction halves and wk/wq are dt-major so the unit-(0,0) critical
    # path (wk dt0 + wq dt0 + x0 c0-3) lands in the first ~4us.
    def load_x(tcc, chalf, eng):
        sl = slice((tcc * CDT + chalf * 4) * TC, (tcc * CDT + chalf * 4 + 4) * TC)
        eng.dma_start(x_sb[:, sl], xT_d[:, sl])

    def load_wdt(w_sb, w_d, dt, eng, chalf=None):
        lo = dt * CDT * P
        hi = (dt + 1) * CDT * P
        if chalf is not None:
            half = (hi - lo) // 2
            lo, hi = lo + chalf * half, lo + (chalf + 1) * half
        sl = slice(lo, hi)
        eng.dma_start(w_sb[:, sl], w_d[:, sl])

    load_x(0, 0, nc.sync)
    load_wdt(wk, wkT_d, 0, nc.gpsimd)
    load_wdt(wq, wqT_d, 0, nc.scalar)
    load_x(0, 1, nc.sync)
    nc.scalar.dma_start(ones_sb[:], ones16_d[:])
    load_x(1, 0, nc.sync)
    load_x(1, 1, nc.scalar)
    nc.gpsimd.dma_start(wv[:], wvT_d[:])
    load_x(2, 0, nc.sync)
    load_x(2, 1, nc.scalar)
    load_x(3, 0, nc.sync)
    load_x(3, 1, nc.scalar)
    for dt in range(1, NDT):
        load_wdt(wk, wkT_d, dt, nc.gpsimd)
        load_wdt(wq, wqT_d, dt, nc.gpsimd)
    nc.scalar.dma_start(wo[:], woT_d[:])

    # ones column of v_sb (one strided copy; constant across the run)
    nc.vector.tensor_copy(
        v_r[:, :, :, HD:HD + 1].rearrange("p k h e -> p (k h e)"),
        ones_sb[:, 0:KT * HG],
    )

    def evict(dst_ap, src_ap, ev):
        # sweep-0 evictions ride the (then-idle) ACT engine: `copy` shares
        # the activation table with `exp`, so no table-load penalty
        if ev == "act":
            nc.scalar.activation(dst_ap, src_ap,
                                 mybir.ActivationFunctionType.Copy)
        else:
            nc.vector.tensor_copy(dst_ap, src_ap)

    def kqproj(w_sb, dst, dt, tcc, ev="dve"):
        _lbl(f"kqproj_{dt}_{tcc}")
        pp = gpool.tile([P, QC], f32, tag="gp")
        for c in range(CDT):
            nc.tensor.matmul(
                pp[:, 0:TC],
                w_sb[:, (dt * CDT + c) * P:(dt * CDT + c + 1) * P],
                x_sb[:, (tcc * CDT + c) * TC:(tcc * CDT + c + 1) * TC],
                start=(c == 0), stop=(c == CDT - 1),
            )
        evict(dst[:, dt * S + tcc * TC:dt * S + (tcc + 1) * TC],
              pp[:, 0:TC], ev)

    def vproj(tt, half, ev="dve"):
        # x stationary: [128 tokens of tt, 256 v-dims = heads 4half..4half+3]
        _lbl(f"vproj_{tt}_{half}")
        pp = gpool.tile([P, QC], f32, tag="gp")
        for c in range(CDT):
            nc.tensor.matmul(
                pp[:, 0:2 * P],
                x_sb[:, (tt // 4 * CDT + c) * TC + (tt % 4) * P:
                         (tt // 4 * CDT + c) * TC + (tt % 4 + 1) * P],
                wv[:].rearrange("p (d c o) -> p d c o", d=NDT, c=CDT)[
                    :, 2 * half:2 * half + 2, c, :],
                start=(c == 0), stop=(c == CDT - 1),
            )
        evict(v_r[:, tt, 4 * half:4 * half + 4, 0:HD],
              pp[:, 0:2 * P].rearrange("p (h d) -> p h d", h=4), ev)

    def scores(h, qc, kt, exps, sps):
        # one matmul per k-tile into alternating banks of a [128, 1024]
        # psum tile; after the odd k-tile, one fused [128, 1024] exp on ACT
        # (half the ACT instruction overhead of per-k-tile exps)
        _lbl(f"scores_{h}_{qc}_{kt}")
        hp, ho = h // 2, (h % 2) * HD
        j, half = kt // 2, kt % 2
        if half == 0:
            sps[j] = spool.tile([P, 2 * QC], f32, tag="sp", name=f"sp_{j}")
        sp = sps[j]
        nc.tensor.matmul(
            sp[:, half * QC:(half + 1) * QC],
            kT[ho:ho + HD, hp * S + kt * P:hp * S + (kt + 1) * P],
            qT[ho:ho + HD, hp * S + qc * QC:hp * S + (qc + 1) * QC],
            start=True, stop=True,
        )
        if half == 1:
            ex = epool.tile([P, 2 * QC], bf16, tag="exp")
            nc.scalar.activation(
                ex[:], sp[:], mybir.ActivationFunctionType.Exp,
                scale=EXP_SCALE)
            exps[j] = ex

    def attnv_half(h, exps, oes, key, hb):
        # 16-matmul accumulation over all k, for one 256-wide q half-chunk.
        # Short (1.7us) atomic groups keep score matmuls flowing so ACT
        # never starves; [65, 256] accumulators pack four to two banks.
        _lbl(f"attnv_{key[0]}_{key[1]}_{hb}")
        oe = apool.tile([P, QC // 2], f32, tag="oe")
        oes[(key, hb)] = oe
        for kt in range(KT):
            j, half = kt // 2, kt % 2
            nc.tensor.matmul(
                oe[0:VE, :],
                v_sb[:, (kt * HG + h) * VE:(kt * HG + h + 1) * VE],
                exps[j][:, half * QC + hb * (QC // 2):
                          half * QC + (hb + 1) * (QC // 2)],
                start=(kt == 0), stop=(kt == KT - 1),
            )

    def div_pre(oes, recrs, key):
        _lbl(f"divpre_{key[0]}_{key[1]}")
        oa, ob = oes[(key, 0)], oes[(key, 1)]
        den = rpool.tile([1, QC], f32, tag="den")
        nc.vector.tensor_copy(den[:, 0:QC // 2], oa[HD:HD + 1, :])
        nc.vector.tensor_copy(den[:, QC // 2:], ob[HD:HD + 1, :])
        rec = rpool.tile([1, QC], f32, tag="rec")
        scr = rpool.tile([1, QC], f32, tag="scr")
        nc.vector.reciprocal_approx_accurate(rec[:], den[:], scr[:])
        recrs[key] = rec

    def div_post(h, qc, oes, recrs, key):
        _lbl(f"divpost_{h}_{qc}")
        oa = oes.pop((key, 0))
        ob = oes.pop((key, 1))
        rec = recrs.pop(key)
        rb = rpool.tile([HD, QC], f32, tag="rb")
        nc.gpsimd.partition_broadcast(rb[:], rec[:])
        hp, ho = h // 2, (h % 2) * HD
        base = hp * S + qc * QC
        nc.vector.tensor_mul(
            outT[ho:ho + HD, base:base + QC // 2],
            oa[0:HD, :], rb[:, 0:QC // 2])
        nc.vector.tensor_mul(
            outT[ho:ho + HD, base + QC // 2:base + QC],
            ob[0:HD, :], rb[:, QC // 2:])

    osbs = {}

    def oproj(tt, half):
        _lbl(f"oproj_{tt}_{half}")
        po = gpool.tile([P, QC], f32, tag="gp")
        for d in range(NDT):
            nc.tensor.matmul(
                po[:],
                outT[:, d * S + tt * P:d * S + (tt + 1) * P],
                wo[:, d * D + half * QC:d * D + (half + 1) * QC],
                start=(d == 0), stop=(d == NDT - 1),
            )
        if tt not in osbs:
            osbs[tt] = ospool.tile([P, D], f32, tag="osb", name=f"osb_{tt}")
        osb = osbs[tt]
        nc.vector.tensor_copy(osb[:, half * QC:(half + 1) * QC], po[:])
        if half == 1:
            nc.sync.dma_start(out_d[tt * P:(tt + 1) * P, :], osb[:])
            del osbs[tt]

    # ---------- schedule ----------
    # prologue: deps of unit (h0, qc0) kt 0-3 + x0-dependent vprojs
    kqproj(wk, kT, 0, 0, "act")
    kqproj(wq, qT, 0, 0, "act")
    for tt in range(4):
        vproj(tt, 0, "act")

    # Static filler assignment per unit index (32 units: qc-major, h-minor).
    # attnv runs with lag 2 (lag 1 in the last sweep to shorten the tail);
    # div_pre rides with attnv, div_post one unit later.  Deadlines:
    # kproj(dt,tc) before scores(2dt, qc0, kt=4tc); qproj(dt,qc) before unit
    # 8qc+2dt; vproj(*,half) before attnv(4half, qc0).
    # filler items are (fn, deadline_kt): deadline_kt forces the item to
    # drain before scores(kt=deadline_kt) of its unit is emitted
    fillers = {i: [] for i in range(33)}

    def push(i, fn, dl=None):
        fillers[min(i, 32)].append((fn, dl))

    def kq(w, dst, dt, tcc, ev="dve"):
        return lambda: kqproj(w, dst, dt, tcc, ev)

    def vp(tt, half, ev="dve"):
        return lambda: vproj(tt, half, ev)

    # unit 0 must finish every vproj(*, 0): attnv(0,0) consumes the full
    # v half at unit 1.  x3-dependent items (kproj(0,3), vp 12-15) go late.
    push(0, kq(wk, kT, 0, 1, "act"), 4)
    for tt in range(4, 8):
        push(0, vp(tt, 0, "act"))
    push(0, kq(wk, kT, 0, 2, "act"), 8)
    for tt in range(8, 12):
        push(0, vp(tt, 0, "act"))
    push(0, kq(wk, kT, 0, 3, "act"), 12)
    for tt in range(12, KT):
        push(0, vp(tt, 0, "act"))
    push(1, kq(wk, kT, 1, 0, "act"))
    push(1, kq(wq, qT, 1, 0, "act"))
    for tcc in range(1, NTC):
        push(2, kq(wk, kT, 1, tcc, "act"), 4 * tcc)
    for tt in range(0, 4):
        push(2, vp(tt, 1, "act"))
    push(3, kq(wk, kT, 2, 0, "act"))
    push(3, kq(wq, qT, 2, 0, "act"))
    for tt in range(4, 10):
        push(3, vp(tt, 1, "act"))
    for tcc in range(1, NTC):
        push(4, kq(wk, kT, 2, tcc, "act"), 4 * tcc)
    for tt in range(10, KT):
        push(4, vp(tt, 1, "act"))
    push(5, kq(wk, kT, 3, 0, "act"))
    push(5, kq(wq, qT, 3, 0, "act"))
    for tcc in range(1, NTC):
        push(6, kq(wk, kT, 3, tcc, "act"), 4 * tcc)
    # q projections for later sweeps: spread over the preceding sweep
    for tcq in range(1, NQC):
        for dt in range(NDT):
            push(8 * (tcq - 1) + 2 * dt + 3, kq(wq, qT, dt, tcq))
    # out-projections for sweep qc (qc3 handled in the tail): outT(qc) is
    # complete after div_post(7,qc), which lands in unit 8qc+10
    for qc in range(NQC - 1):
        for tt4 in range(4):
            tt = qc * 4 + tt4
            for half in range(2):
                push(8 * qc + 10 + tt4,
                     (lambda t, hf: lambda: oproj(t, hf))(tt, half))

    oes, recrs = {}, {}
    unit = 0
    for qc in range(NQC):
        for h in range(HG):
            exps = [None] * (KT // 2)
            sps = [None] * (KT // 2)
            todo = list(fillers[unit])
            n = len(todo)
            done = 0
            for ktg in range(KT // 4):
                kt0 = 4 * ktg
                want = (kt0 + 4) * n // KT
                for j in range(done, n):
                    if todo[j][1] is not None and todo[j][1] <= kt0 + 3:
                        want = max(want, j + 1)
                while done < want:
                    todo[done][0]()
                    done += 1
                # 4-score burst (2 psum tiles): trailing LDWEIGHTS prefetch
                # under the preceding matmuls instead of exposing
                for kt in range(kt0, kt0 + 4):
                    scores(h, qc, kt, exps, sps)
            while done < n:
                todo[done][0]()
                done += 1
            key = (h, qc)
            if unit < 31:
                dst = fillers[unit + 1]
                dst.insert(min(2, len(dst)),
                           ((lambda e, kk: lambda: attnv_half(kk[0], e, oes, kk, 0))(exps, key), None))
                dst.insert(min(3 + len(dst) // 2, len(dst)),
                           ((lambda e, kk: lambda: attnv_half(kk[0], e, oes, kk, 1))(exps, key), None))
                push(unit + 1, (lambda kk: lambda: div_pre(oes, recrs, kk))(key))
                fillers[min(unit + 2, 32)].insert(
                    0, ((lambda kk: lambda: div_post(kk[0], kk[1], oes, recrs, kk))(key), None))
            else:
                # last unit: k-split halves so half A runs here, half B +
                # division + out-projections form a minimal serial tail
                _lbl("attnv_7_3_ka")
                oa31 = gpool.tile([P, QC], f32, tag="gp", name="oa31")
                for kt in range(KT // 2):
                    nc.tensor.matmul(
                        oa31[0:VE, :],
                        v_sb[:, (kt * HG + h) * VE:(kt * HG + h + 1) * VE],
                        exps[kt // 2][:, (kt % 2) * QC:(kt % 2 + 1) * QC],
                        start=(kt == 0), stop=(kt == KT // 2 - 1),
                    )
                exps31, key31 = exps, key
            unit += 1
    # tail: drain leftovers (div_post(30) etc.), then the last unit's
    # attn@v + division, then the final sweep's out-projections
    for fn, _ in fillers[32]:
        fn()
    _lbl("attnv_7_3_kb")
    h31 = key31[0]
    ob31 = gpool.tile([P, QC], f32, tag="gp", name="ob31")
    for i in range(KT // 2):
        kt = KT // 2 + i
        nc.tensor.matmul(
            ob31[0:VE, :],
            v_sb[:, (kt * HG + h31) * VE:(kt * HG + h31 + 1) * VE],
            exps31[kt // 2][:, (kt % 2) * QC:(kt % 2 + 1) * QC],
            start=(i == 0), stop=(i == KT // 2 - 1),
        )
    _lbl("div_7_3")
    # DVE may read only one PSUM operand per instruction: stage half A
    dena = rpool.tile([1, QC], f32, tag="dena")
    nc.vector.tensor_copy(dena[:], oa31[HD:HD + 1, :])
    den = rpool.tile([1, QC], f32, tag="den")
    nc.vector.tensor_add(den[:], dena[:], ob31[HD:HD + 1, :])
    rec = rpool.tile([1, QC], f32, tag="rec")
    scr = rpool.tile([1, QC], f32, tag="scr")
    nc.vector.reciprocal_approx_accurate(rec[:], den[:], scr[:])
    rb = rpool.tile([HD, QC], f32, tag="rb")
    nc.gpsimd.partition_broadcast(rb[:], rec[:])
    hp31, ho31 = h31 // 2, (h31 % 2) * HD
    base31 = hp31 * S + key31[1] * QC
    ot31 = outT[ho31:ho31 + HD, base31:base31 + QC]
    tmp31 = rpool.tile([P, QC], f32, tag="osum")
    t31 = tmp31[ho31:ho31 + HD, :]
    nc.vector.tensor_mul(ot31, oa31[0:HD, :], rb[:])
    nc.vector.tensor_mul(t31, ob31[0:HD, :], rb[:])
    nc.vector.tensor_add(ot31, ot31, t31)
    for tt in range(12, KT):
        for half in range(2):
            oproj(tt, half)


def _build():
    global _CACHED_NC
    if _CACHED_NC is not None:
        return _CACHED_NC
    nc = bacc.Bacc("TRN2", target_bir_lowering=False, debug=False)
    xT = nc.dram_tensor("xT", [P, NTC * CDT * TC], bf16,
                        kind="ExternalInput").ap()
    wqT = nc.dram_tensor("wqT", [P, CDT * GD], bf16, kind="ExternalInput").ap()
    wkT = nc.dram_tensor("wkT", [P, CDT * GD], bf16, kind="ExternalInput").ap()
    wvT = nc.dram_tensor("wvT", [P, CDT * GD], bf16, kind="ExternalInput").ap()
    woT = nc.dram_tensor("woT", [P, NDT * D], bf16, kind="ExternalInput").ap()
    ones16 = nc.dram_tensor("ones16", [P, P], bf16, kind="ExternalInput").ap()
    out = nc.dram_tensor("out", [S, D], f32, kind="ExternalOutput").ap()

    with tile.TileContext(nc) as tcx:
        _mha_kernel(tcx, [xT, wqT, wkT, wvT, woT, ones16], [out])
    nc.compile()
    _CACHED_NC = nc
    return nc


def kernel(x: np.ndarray, Wq: np.ndarray, Wk: np.ndarray, Wv: np.ndarray,
           Wo: np.ndarray) -> np.ndarray:
    global LAST_EXEC_TIME_NS
    import ml_dtypes
    bf = np.dtype(ml_dtypes.bfloat16)
    nc = _build()

    x = np.asarray(x, dtype=np.float32)
    ones16 = np.ones((P, P), dtype=bf)

    def x_lin(xb):
        # [p, (tc, c, t)]: xb[tc*512+t, c*128+p]
        return np.ascontiguousarray(
            xb.reshape(NTC, TC, CDT, P).transpose(3, 0, 2, 1).reshape(P, -1)
        ).astype(bf)

    def w_lin(Wm, g):
        # [p, (dt, c, o128)]: W[g*GD + dt*128 + o, c*128 + p]
        ws = np.asarray(Wm, np.float32)[g * GD:(g + 1) * GD, :]
        ws = ws.reshape(NDT, P, CDT, P)          # [dt, o, c, p]
        return np.ascontiguousarray(
            ws.transpose(3, 0, 2, 1).reshape(P, -1)
        ).astype(bf)

    def wo_lin(Wm, g):
        # [p, (d, e)]: Wo[e, g*GD + d*128+p]
        ws = np.asarray(Wm, np.float32)[:, g * GD:(g + 1) * GD]
        return np.ascontiguousarray(
            ws.reshape(D, NDT, P).transpose(2, 1, 0).reshape(P, -1)
        ).astype(bf)

    in_maps = []
    for c in range(NCORES):
        b, g = c // NG, c % NG
        in_maps.append({
            "xT": x_lin(x[b]),
            "wqT": w_lin(Wq, g),
            "wkT": w_lin(Wk, g),
            "wvT": w_lin(Wv, g),
            "woT": wo_lin(Wo, g),
            "ones16": ones16,
        })

    trace = bool(os.environ.get("BASS_TRACE"))
    res = run_bass_kernel_spmd(nc, in_maps, core_ids=list(range(NCORES)),
                               trace=trace)
    LAST_EXEC_TIME_NS = res.exec_time_ns

    out = np.empty((B, S, D), dtype=np.float32)
    for b in range(B):
        out[b] = res.results[NG * b]["out"].astype(np.float32)
        for g in range(1, NG):
            out[b] += res.results[NG * b + g]["out"]
    return out


# revision 39
# speedup vs baseline: 1.0085x; 1.0085x over previous
"""Multi-head attention (B=4, S=2048, D=1024, H=16) on 8 trn2 NeuronCores.

Sharding: B x H grid -- core c owns batch c//2 and head-group c%2 (8 heads
= 512 model dims).  Each core computes q/k/v projections for its 512 dims
over its batch's 2048 tokens, local attention for its 8 heads, and a
partial out-projection against its 512 rows of Wo.  Host sums the two
partial outputs per batch (the tensor-parallel all-reduce, done at gather
time) and stacks batches.  vs head-only sharding this cuts per-core DMA
from ~69MB to ~16MB; on-chip FLOPs are identical.

Per-core kernel (bf16 operands, fp32 PSUM):
  x_sb   [128, 8, 2048]  x^T (contraction dim on partitions), bf16 from host.
  q/k    [128, 4, 2048]  dim-major (head h = dim-tile h//2, rows (h%2)*64+).
  v_sb   [128, 16, 8, 65] token-major (128 tokens per k-tile on partitions)
         computed directly by x-stationary projection matmuls -- no PE
         transposes -- with a constant ones column per head so attn@v gives
         numerator + softmax denominator in one accumulation group.
  scores [128 k, 512 q] one matmul per (head, ktile, qchunk); exp on ACT
         (scale=1/8 fused) straight to bf16 SBUF tiles.
  division: DVE reciprocal of the denominator row, broadcast across 64
         partitions via a 1-row f32r matmul, DVE multiply into outT.
  oproj  outT [128, 4, 2048] @ wo_sb -> [128 tok, 1024] psum, DVE evict,
         DMA out fp32.

Scheduling: ACT exp (~280us) and PE streaming (~330us) are the co-rooflines;
everything is emitted as self-contained PE work units (projection groups,
attn@v accumulation groups, out-proj groups) hand-paced between the score
matmuls so the PE never idles (p-state stays 2.4GHz) while ACT stays fed
through a 4-deep PSUM score window.  PSUM budget: 4 score banks + 2 attn@v
accumulators + 2 rotating gemm banks = 8.
"""
import os
import sys

sys.path.insert(0, "/opt/trn_rl_repo")

import numpy as np

import concourse.mybir as mybir
import concourse.tile as tile
from concourse import bacc
from concourse._compat import with_exitstack
from concourse.bass_utils import run_bass_kernel_spmd
from contextlib import ExitStack

B, S, D, H = 4, 2048, 1024, 16
HD = D // H              # 64
P = 128
NCORES = 8
NG = 2                   # head-groups (tensor-parallel degree)
HG = H // NG             # 8 heads per core
GD = HG * HD             # 512 group dims
NDT = GD // P            # 4 dim-tiles of q/k/v per core
CDT = D // P             # 8 contraction tiles
KT = S // P              # 16 k-tiles
QC = 512                 # q-chunk / matmul free dim (one psum bank)
NQC = S // QC            # 4
TC = 512                 # projection token chunk
NTC = S // TC            # 4
VE = HD + 1              # v + ones column
EXP_SCALE = float(1.0 / np.sqrt(HD))

f32 = mybir.dt.float32
f32r = mybir.dt.float32r
bf16 = mybir.dt.bfloat16

LAST_EXEC_TIME_NS = None
_CACHED_NC = None
CURRENT_LABEL = ["init"]


def _lbl(s):
    CURRENT_LABEL[0] = s


@with_exitstack
def _mha_kernel(ctx: ExitStack, tc_: tile.TileContext, ins, outs):
    nc = tc_.nc
    xT_d, wqT_d, wkT_d, wvT_d, woT_d, ones16_d = ins
    out_d = outs[0]

    const = ctx.enter_context(tc_.tile_pool(name="const", bufs=1))
    wpool = ctx.enter_context(tc_.tile_pool(name="wpool", bufs=1))
    xpool = ctx.enter_context(tc_.tile_pool(name="xpool", bufs=1))
    qkpool = ctx.enter_context(tc_.tile_pool(name="qkpool", bufs=1))
    vpool = ctx.enter_context(tc_.tile_pool(name="vpool", bufs=1))
    epool = ctx.enter_context(tc_.tile_pool(name="epool", bufs=20))
    rpool = ctx.enter_context(tc_.tile_pool(name="rpool", bufs=2))
    ospool = ctx.enter_context(tc_.tile_pool(name="ospool", bufs=2))

    # PSUM: 2x2 score banks (kt-pair fused exp) + 4 quarter-bank attn@v
    # accumulators (2 banks) + 2 gemm banks = 8
    spool = ctx.enter_context(tc_.tile_pool(name="spool", bufs=2, space="PSUM"))
    apool = ctx.enter_context(tc_.tile_pool(name="apool", bufs=2, space="PSUM"))
    gpool = ctx.enter_context(tc_.tile_pool(name="gpool", bufs=2, space="PSUM"))

    ones_sb = const.tile([P, P], bf16, tag="ones_sb")

    x_sb = xpool.tile([P, CDT * S], bf16, tag="x")
    wq = wpool.tile([P, CDT * GD], bf16, tag="wq")
    wk = wpool.tile([P, CDT * GD], bf16, tag="wk")
    wv = wpool.tile([P, CDT * GD], bf16, tag="wv")
    wo = wpool.tile([P, NDT * D], bf16, tag="wo")
    qT = qkpool.tile([P, NDT * S], bf16, tag="qT")
    kT = qkpool.tile([P, NDT * S], bf16, tag="kT")
    outT = qkpool.tile([P, NDT * S], bf16, tag="outT")
    v_sb = vpool.tile([P, KT * HG * VE], bf16, tag="v")
    v_r = v_sb[:].rearrange("p (k h e) -> p k h e", k=KT, h=HG)

    # All DRAM tensors are host-prearranged SBUF partition images: every
    # DMA is 128 contiguous descriptors.  x chunks are split into
    # contraction halves and wk/wq are dt-major so the unit-(0,0) critical
    # path (wk dt0 + wq dt0 + x0 c0-3) lands in the first ~4us.
    def load_x(tcc, chalf, eng):
        sl = slice((tcc * CDT + chalf * 4) * TC, (tcc * CDT + chalf * 4 + 4) * TC)
        eng.dma_start(x_sb[:, sl], xT_d[:, sl])

    def load_wdt(w_sb, w_d, dt, eng, chalf=None):
        lo = dt * CDT * P
        hi = (dt + 1) * CDT * P
        if chalf is not None:
            half = (hi - lo) // 2
            lo, hi = lo + chalf * half, lo + (chalf + 1) * half
        sl = slice(lo, hi)
        eng.dma_start(w_sb[:, sl], w_d[:, sl])

    load_wdt(wk, wkT_d, 0, nc.gpsimd, 0)
    load_x(0, 0, nc.sync)
    load_wdt(wk, wkT_d, 0, nc.gpsimd, 1)
    load_wdt(wq, wqT_d, 0, nc.scalar)
    load_x(0, 1, nc.sync)
    nc.scalar.dma_start(ones_sb[:], ones16_d[:])
    load_x(1, 0, nc.sync)
    load_x(1, 1, nc.scalar)
    nc.gpsimd.dma_start(wv[:], wvT_d[:])
    load_x(2, 0, nc.sync)
    load_x(2, 1, nc.scalar)
    load_x(3, 0, nc.sync)
    load_x(3, 1, nc.scalar)
    for dt in range(1, NDT):
        load_wdt(wk, wkT_d, dt, nc.gpsimd)
        load_wdt(wq, wqT_d, dt, nc.gpsimd)
    nc.scalar.dma_start(wo[:], woT_d[:])

    # ones column of v_sb (one strided copy; constant across the run)
    nc.vector.tensor_copy(
        v_r[:, :, :, HD:HD + 1].rearrange("p k h e -> p (k h e)"),
        ones_sb[:, 0:KT * HG],
    )

    def evict(dst_ap, src_ap, ev):
        # sweep-0 evictions ride the (then-idle) ACT engine: `copy` shares
        # the activation table with `exp`, so no table-load penalty
        if ev == "act":
            nc.scalar.activation(dst_ap, src_ap,
                                 mybir.ActivationFunctionType.Copy)
        else:
            nc.vector.tensor_copy(dst_ap, src_ap)

    def kqproj(w_sb, dst, dt, tcc, ev="dve"):
        _lbl(f"kqproj_{dt}_{tcc}")
        pp = gpool.tile([P, QC], f32, tag="gp")
        for c in range(CDT):
            nc.tensor.matmul(
                pp[:, 0:TC],
                w_sb[:, (dt * CDT + c) * P:(dt * CDT + c + 1) * P],
                x_sb[:, (tcc * CDT + c) * TC:(tcc * CDT + c + 1) * TC],
                start=(c == 0), stop=(c == CDT - 1),
            )
        evict(dst[:, dt * S + tcc * TC:dt * S + (tcc + 1) * TC],
              pp[:, 0:TC], ev)

    def vproj(tt, half, ev="dve"):
        # x stationary: [128 tokens of tt, 256 v-dims = heads 4half..4half+3]
        _lbl(f"vproj_{tt}_{half}")
        pp = gpool.tile([P, QC], f32, tag="gp")
        for c in range(CDT):
            nc.tensor.matmul(
                pp[:, 0:2 * P],
                x_sb[:, (tt // 4 * CDT + c) * TC + (tt % 4) * P:
                         (tt // 4 * CDT + c) * TC + (tt % 4 + 1) * P],
                wv[:].rearrange("p (d c o) -> p d c o", d=NDT, c=CDT)[
                    :, 2 * half:2 * half + 2, c, :],
                start=(c == 0), stop=(c == CDT - 1),
            )
        evict(v_r[:, tt, 4 * half:4 * half + 4, 0:HD],
              pp[:, 0:2 * P].rearrange("p (h d) -> p h d", h=4), ev)

    def scores(h, qc, kt, exps, sps):
        # one matmul per k-tile into alternating banks of a [128, 1024]
        # psum tile; after the odd k-tile, one fused [128, 1024] exp on ACT
        # (half the ACT instruction overhead of per-k-tile exps)
        _lbl(f"scores_{h}_{qc}_{kt}")
        hp, ho = h // 2, (h % 2) * HD
        j, half = kt // 2, kt % 2
        if half == 0:
            sps[j] = spool.tile([P, 2 * QC], f32, tag="sp", name=f"sp_{j}")
        sp = sps[j]
        nc.tensor.matmul(
            sp[:, half * QC:(half + 1) * QC],
            kT[ho:ho + HD, hp * S + kt * P:hp * S + (kt + 1) * P],
            qT[ho:ho + HD, hp * S + qc * QC:hp * S + (qc + 1) * QC],
            start=True, stop=True,
        )
        if half == 1:
            ex = epool.tile([P, 2 * QC], bf16, tag="exp")
            nc.scalar.activation(
                ex[:], sp[:], mybir.ActivationFunctionType.Exp,
                scale=EXP_SCALE)
            exps[j] = ex

    def attnv_half(h, exps, oes, key, hb):
        # 16-matmul accumulation over all k, for one 256-wide q half-chunk.
        # Short (1.7us) atomic groups keep score matmuls flowing so ACT
        # never starves; [65, 256] accumulators pack four to two banks.
        _lbl(f"attnv_{key[0]}_{key[1]}_{hb}")
        oe = apool.tile([P, QC // 2], f32, tag="oe")
        oes[(key, hb)] = oe
        for kt in range(KT):
            j, half = kt // 2, kt % 2
            nc.tensor.matmul(
                oe[0:VE, :],
                v_sb[:, (kt * HG + h) * VE:(kt * HG + h + 1) * VE],
                exps[j][:, half * QC + hb * (QC // 2):
                          half * QC + (hb + 1) * (QC // 2)],
                start=(kt == 0), stop=(kt == KT - 1),
            )

    def div_pre(oes, recrs, key):
        _lbl(f"divpre_{key[0]}_{key[1]}")
        oa, ob = oes[(key, 0)], oes[(key, 1)]
        den = rpool.tile([1, QC], f32, tag="den")
        nc.vector.tensor_copy(den[:, 0:QC // 2], oa[HD:HD + 1, :])
        nc.vector.tensor_copy(den[:, QC // 2:], ob[HD:HD + 1, :])
        rec = rpool.tile([1, QC], f32, tag="rec")
        scr = rpool.tile([1, QC], f32, tag="scr")
        nc.vector.reciprocal_approx_accurate(rec[:], den[:], scr[:])
        recrs[key] = rec

    def div_post(h, qc, oes, recrs, key):
        _lbl(f"divpost_{h}_{qc}")
        oa = oes.pop((key, 0))
        ob = oes.pop((key, 1))
        rec = recrs.pop(key)
        rb = rpool.tile([HD, QC], f32, tag="rb")
        nc.gpsimd.partition_broadcast(rb[:], rec[:])
        hp, ho = h // 2, (h % 2) * HD
        base = hp * S + qc * QC
        nc.vector.tensor_mul(
            outT[ho:ho + HD, base:base + QC // 2],
            oa[0:HD, :], rb[:, 0:QC // 2])
        nc.vector.tensor_mul(
            outT[ho:ho + HD, base + QC // 2:base + QC],
            ob[0:HD, :], rb[:, QC // 2:])

    osbs = {}

    def oproj(tt, half):
        _lbl(f"oproj_{tt}_{half}")
        po = gpool.tile([P, QC], f32, tag="gp")
        for d in range(NDT):
            nc.tensor.matmul(
                po[:],
                outT[:, d * S + tt * P:d * S + (tt + 1) * P],
                wo[:, d * D + half * QC:d * D + (half + 1) * QC],
                start=(d == 0), stop=(d == NDT - 1),
            )
        if tt not in osbs:
            osbs[tt] = ospool.tile([P, D], f32, tag="osb", name=f"osb_{tt}")
        osb = osbs[tt]
        nc.vector.tensor_copy(osb[:, half * QC:(half + 1) * QC], po[:])
        if half == 1:
            nc.sync.dma_start(out_d[tt * P:(tt + 1) * P, :], osb[:])
            del osbs[tt]

    # ---------- schedule ----------
    # prologue: deps of unit (h0, qc0) kt 0-3 + x0-dependent vprojs
    kqproj(wk, kT, 0, 0, "act")
    kqproj(wq, qT, 0, 0, "act")
    for tt in range(4):
        vproj(tt, 0, "act")

    # Static filler assignment per unit index (32 units: qc-major, h-minor).
    # attnv runs with lag 2 (lag 1 in the last sweep to shorten the tail);
    # div_pre rides with attnv, div_post one unit later.  Deadlines:
    # kproj(dt,tc) before scores(2dt, qc0, kt=4tc); qproj(dt,qc) before unit
    # 8qc+2dt; vproj(*,half) before attnv(4half, qc0).
    # filler items are (fn, deadline_kt): deadline_kt forces the item to
    # drain before scores(kt=deadline_kt) of its unit is emitted
    fillers = {i: [] for i in range(33)}

    def push(i, fn, dl=None):
        fillers[min(i, 32)].append((fn, dl))

    def kq(w, dst, dt, tcc, ev="dve"):
        return lambda: kqproj(w, dst, dt, tcc, ev)

    def vp(tt, half, ev="dve"):
        return lambda: vproj(tt, half, ev)

    # unit 0 must finish every vproj(*, 0): attnv(0,0) consumes the full
    # v half at unit 1.  x3-dependent items (kproj(0,3), vp 12-15) go late.
    push(0, kq(wk, kT, 0, 1, "act"), 4)
    for tt in range(4, 8):
        push(0, vp(tt, 0, "act"))
    push(0, kq(wk, kT, 0, 2, "act"), 8)
    for tt in range(8, 12):
        push(0, vp(tt, 0, "act"))
    push(0, kq(wk, kT, 0, 3, "act"), 12)
    for tt in range(12, KT):
        push(0, vp(tt, 0, "act"))
    push(1, kq(wk, kT, 1, 0, "act"))
    push(1, kq(wq, qT, 1, 0, "act"))
    for tcc in range(1, NTC):
        push(2, kq(wk, kT, 1, tcc, "act"), 4 * tcc)
    for tt in range(0, 4):
        push(2, vp(tt, 1, "act"))
    push(3, kq(wk, kT, 2, 0, "act"))
    push(3, kq(wq, qT, 2, 0, "act"))
    for tt in range(4, 10):
        push(3, vp(tt, 1, "act"))
    for tcc in range(1, NTC):
        push(4, kq(wk, kT, 2, tcc, "act"), 4 * tcc)
    for tt in range(10, KT):
        push(4, vp(tt, 1, "act"))
    push(5, kq(wk, kT, 3, 0, "act"))
    push(5, kq(wq, qT, 3, 0, "act"))
    for tcc in range(1, NTC):
        push(6, kq(wk, kT, 3, tcc, "act"), 4 * tcc)
    # q projections for later sweeps: spread over the preceding sweep
    for tcq in range(1, NQC):
        for dt in range(NDT):
            push(8 * (tcq - 1) + 2 * dt + 3, kq(wq, qT, dt, tcq))
    # out-projections for sweep qc (qc3 handled in the tail): outT(qc) is
    # complete after div_post(7,qc), which lands in unit 8qc+10
    for qc in range(NQC - 1):
        for tt4 in range(4):
            tt = qc * 4 + tt4
            for half in range(2):
                push(8 * qc + 10 + tt4,
                     (lambda t, hf: lambda: oproj(t, hf))(tt, half))

    oes, recrs = {}, {}
    unit = 0
    for qc in range(NQC):
        for h in range(HG):
            exps = [None] * (KT // 2)
            sps = [None] * (KT // 2)
            todo = list(fillers[unit])
            n = len(todo)
            done = 0
            for ktg in range(KT // 4):
                kt0 = 4 * ktg
                want = (kt0 + 4) * n // KT
                for j in range(done, n):
                    if todo[j][1] is not None and todo[j][1] <= kt0 + 3:
                        want = max(want, j + 1)
                while done < want:
                    todo[done][0]()
                    done += 1
                # 4-score burst (2 psum tiles): trailing LDWEIGHTS prefetch
                # under the preceding matmuls instead of exposing
                for kt in range(kt0, kt0 + 4):
                    scores(h, qc, kt, exps, sps)
            while done < n:
                todo[done][0]()
                done += 1
            key = (h, qc)
            if unit < 31:
                dst = fillers[unit + 1]
                dst.insert(min(1, len(dst)),
                           ((lambda e, kk: lambda: attnv_half(kk[0], e, oes, kk, 0))(exps, key), None))
                dst.insert(min(2 + len(dst) // 2, len(dst)),
                           ((lambda e, kk: lambda: attnv_half(kk[0], e, oes, kk, 1))(exps, key), None))
                push(unit + 1, (lambda kk: lambda: div_pre(oes, recrs, kk))(key))
                fillers[min(unit + 2, 32)].insert(
                    0, ((lambda kk: lambda: div_post(kk[0], kk[1], oes, recrs, kk))(key), None))
            else:
                # last unit: k-split halves so half A runs here, half B +
                # division + out-projections form a minimal serial tail
                _lbl("attnv_7_3_ka")
                oa31 = gpool.tile([P, QC], f32, tag="gp", name="oa31")
                for kt in range(KT // 2):
                    nc.tensor.matmul(
                        oa31[0:VE, :],
                        v_sb[:, (kt * HG + h) * VE:(kt * HG + h + 1) * VE],
                        exps[kt // 2][:, (kt % 2) * QC:(kt % 2 + 1) * QC],
                        start=(kt == 0), stop=(kt == KT // 2 - 1),
                    )
                exps31, key31 = exps, key
            unit += 1
    # tail: drain leftovers (div_post(30) etc.), then the last unit's
    # attn@v + division, then the final sweep's out-projections
    for fn, _ in fillers[32]:
        fn()
    _lbl("attnv_7_3_kb")
    h31 = key31[0]
    ob31 = gpool.tile([P, QC], f32, tag="gp", name="ob31")
    for i in range(KT // 2):
        kt = KT // 2 + i
        nc.tensor.matmul(
            ob31[0:VE, :],
            v_sb[:, (kt * HG + h31) * VE:(kt * HG + h31 + 1) * VE],
            exps31[kt // 2][:, (kt % 2) * QC:(kt % 2 + 1) * QC],
            start=(i == 0), stop=(i == KT // 2 - 1),
        )
    _lbl("div_7_3")
    # DVE may read only one PSUM operand per instruction: stage half A
    dena = rpool.tile([1, QC], f32, tag="dena")
    nc.vector.tensor_copy(dena[:], oa31[HD:HD + 1, :])
    den = rpool.tile([1, QC], f32, tag="den")
    nc.vector.tensor_add(den[:], dena[:], ob31[HD:HD + 1, :])
    rec = rpool.tile([1, QC], f32, tag="rec")
    scr = rpool.tile([1, QC], f32, tag="scr")
    nc.vector.reciprocal_approx_accurate(rec[:], den[:], scr[:])
    rb = rpool.tile([HD, QC], f32, tag="rb")
    nc.gpsimd.partition_broadcast(rb[:], rec[:])
    hp31, ho31 = h31 // 2, (h31 % 2) * HD
    base31 = hp31 * S + key31[1] * QC
    ot31 = outT[ho31:ho31 + HD, base31:base31 + QC]
    tmp31 = rpool.tile([P, QC], f32, tag="osum")
    t31 = tmp31[ho31:ho31 + HD, :]
    nc.vector.tensor_mul(ot31, oa31[0:HD, :], rb[:])
    nc.vector.tensor_mul(t31, ob31[0:HD, :], rb[:])
    nc.vector.tensor_add(ot31, ot31, t31)
    for tt in range(12, KT):
        for half in range(2):
            oproj(tt, half)


def _build():
    global _CACHED_NC
    if _CACHED_NC is not None:
        return _CACHED_NC
    nc = bacc.Bacc("TRN2", target_bir_lowering=False, debug=False)
    xT = nc.dram_tensor("xT", [P, NTC * CDT * TC], bf16,
                        kind="ExternalInput").ap()
    wqT = nc.dram_tensor("wqT", [P, CDT * GD], bf16, kind="ExternalInput").ap()
    wkT = nc.dram_tensor("wkT", [P, CDT * GD], bf16, kind="ExternalInput").ap()
    wvT = nc.dram_tensor("wvT", [P, CDT * GD], bf16, kind="ExternalInput").ap()
    woT = nc.dram_tensor("woT", [P, NDT * D], bf16, kind="ExternalInput").ap()
    ones16 = nc.dram_tensor("ones16", [P, P], bf16, kind="ExternalInput").ap()
    out = nc.dram_tensor("out", [S, D], f32, kind="ExternalOutput").ap()

    with tile.TileContext(nc) as tcx:
        _mha_kernel(tcx, [xT, wqT, wkT, wvT, woT, ones16], [out])
    nc.compile()
    _CACHED_NC = nc
    return nc


def kernel(x: np.ndarray, Wq: np.ndarray, Wk: np.ndarray, Wv: np.ndarray,
           Wo: np.ndarray) -> np.ndarray:
    global LAST_EXEC_TIME_NS
    import ml_dtypes
    bf = np.dtype(ml_dtypes.bfloat16)
    nc = _build()

    x = np.asarray(x, dtype=np.float32)
    ones16 = np.ones((P, P), dtype=bf)

    def x_lin(xb):
        # [p, (tc, c, t)]: xb[tc*512+t, c*128+p]
        return np.ascontiguousarray(
            xb.reshape(NTC, TC, CDT, P).transpose(3, 0, 2, 1).reshape(P, -1)
        ).astype(bf)

    def w_lin(Wm, g):
        # [p, (dt, c, o128)]: W[g*GD + dt*128 + o, c*128 + p]
        ws = np.asarray(Wm, np.float32)[g * GD:(g + 1) * GD, :]
        ws = ws.reshape(NDT, P, CDT, P)          # [dt, o, c, p]
        return np.ascontiguousarray(
            ws.transpose(3, 0, 2, 1).reshape(P, -1)
        ).astype(bf)

    def wo_lin(Wm, g):
        # [p, (d, e)]: Wo[e, g*GD + d*128+p]
        ws = np.asarray(Wm, np.float32)[:, g * GD:(g + 1) * GD]
        return np.ascontiguousarray(
            ws.reshape(D, NDT, P).transpose(2, 1, 0).reshape(P, -1)
        ).astype(bf)

    in_maps = []
    for c in range(NCORES):
        b, g = c // NG, c % NG
        in_maps.append({
            "xT": x_lin(x[b]),
            "wqT": w_lin(Wq, g),
            "wkT": w_lin(Wk, g),
            "wvT": w_lin(Wv, g),
            "woT": wo_lin(Wo, g),
            "ones16": ones16,
        })

    trace = bool(os.environ.get("BASS_TRACE"))
    res = run_bass_kernel_spmd(nc, in_maps, core_ids=list(range(NCORES)),
                               trace=trace)
    LAST_EXEC_TIME_NS = res.exec_time_ns

    out = np.empty((B, S, D), dtype=np.float32)
    for b in range(B):
        out[b] = res.results[NG * b]["out"].astype(np.float32)
        for g in range(1, NG):
            out[b] += res.results[NG * b + g]["out"]
    return out
